# revision 2
# baseline (speedup 1.0000x reference)
"""Squeeze-and-Excitation attention module on 8 Trainium2 NeuronCores.

Reference computation (per image b):
    y[c]  = mean(x[b, c, :, :])                      # global average pool
    z     = relu(w1 @ y + b1)                        # FC 512 -> 32
    s     = sigmoid(w2 @ z + b2)                     # FC 32 -> 512
    out[b, c, :, :] = x[b, c, :, :] * s[c]

Sharding: data-parallel over batch. 32 images / 8 cores = 4 images per
core; the tiny FC weights are replicated. Each core streams its images
through SBUF once (load -> pool -> FCs -> in-place scale -> store), so
HBM traffic is the 2x33.5 MB minimum per core.

Layouts (prepared host-side, all free for 68 KB of weights):
    x      [4, 512, 4096]  per-core shard, spatial flattened
    w1t    [128, 4, 32]    w1t[p, k, r] = w1[r, 128k + p]
    b1     [32, 1]
    w2t    [32, 4, 128]    w2t[r, k, p] = w2[128k + p, r]
    b2c    [128, 4]        b2c[p, k]   = b2[128k + p]

On-core pipeline per image (channels split into 4 chunks of 128):
    DMA 8 MB  x[b] -> SBUF tile [128, 4, 4096]
    DVE       row-sums -> [128, 4]
    PE        4 accumulating matmuls -> z_psum[32, 1]
    ACT       relu(z/4096 + b1) -> z
    PE        4 matmuls -> s_psum[128, 4]
    ACT       sigmoid(s + b2) per chunk, then in-place x *= s
    DMA 8 MB  SBUF -> out[b]
"""

import numpy as np

B = 32
C = 512
HW = 64 * 64
N_CORES = 8
B_LOC = B // N_CORES
KC = C // 128  # channel chunks of 128

_NC_CACHE = {}

# Set by test harness to capture a profile; harmless default for grading.
TRACE = False
LAST_RESULT = None


def _build_nc():
    from contextlib import ExitStack

    import concourse.tile as tile
    from concourse import bacc, mybir

    f32 = mybir.dt.float32
    nc = bacc.Bacc("TRN2", target_bir_lowering=False, debug=False)

    x = nc.dram_tensor("x", [B_LOC, C, HW], f32, kind="ExternalInput")
    w1t = nc.dram_tensor("w1t", [128, KC, 32], f32, kind="ExternalInput")
    b1 = nc.dram_tensor("b1", [32, 1], f32, kind="ExternalInput")
    w2t = nc.dram_tensor("w2t", [32, KC, 128], f32, kind="ExternalInput")
    b2c = nc.dram_tensor("b2c", [128, KC], f32, kind="ExternalInput")
    out = nc.dram_tensor("out", [B_LOC, C, HW], f32, kind="ExternalOutput")

    with ExitStack() as ctx:
        tc = ctx.enter_context(tile.TileContext(nc))
        singles = ctx.enter_context(tc.tile_pool(name="singles", bufs=1))
        xpool = ctx.enter_context(tc.tile_pool(name="xpool", bufs=2))
        small = ctx.enter_context(tc.tile_pool(name="small", bufs=4))
        psum = ctx.enter_context(tc.tile_pool(name="psum", bufs=2, space="PSUM"))

        w1t_sb = singles.tile([128, KC, 32], f32)
        nc.sync.dma_start(out=w1t_sb, in_=w1t[:])
        b1_sb = singles.tile([32, 1], f32)
        nc.sync.dma_start(out=b1_sb, in_=b1[:])
        w2t_sb = singles.tile([32, KC, 128], f32)
        nc.sync.dma_start(out=w2t_sb, in_=w2t[:])
        b2_sb = singles.tile([128, KC], f32)
        nc.sync.dma_start(out=b2_sb, in_=b2c[:])

        for b in range(B_LOC):
            xt = xpool.tile([128, KC, HW], f32, tag="x")
            nc.sync.dma_start(out=xt, in_=x[b].rearrange("(k p) j -> p k j", p=128))

            sums = small.tile([128, KC], f32, tag="sums")
            nc.vector.tensor_reduce(
                out=sums, in_=xt, axis=mybir.AxisListType.X, op=mybir.AluOpType.add
            )

            zp = psum.tile([32, 1], f32, tag="z")
            for k in range(KC):
                nc.tensor.matmul(
                    zp,
                    lhsT=w1t_sb[:, k, :],
                    rhs=sums[:, k : k + 1],
                    start=(k == 0),
                    stop=(k == KC - 1),
                )
            z = small.tile([32, 1], f32, tag="z_sb")
            nc.scalar.activation(
                z, zp, mybir.ActivationFunctionType.Relu, bias=b1_sb, scale=1.0 / HW
            )

            sp = psum.tile([128, KC], f32, tag="s")
            for k in range(KC):
                nc.tensor.matmul(
                    sp[:, k : k + 1],
                    lhsT=w2t_sb[:, k, :],
                    rhs=z,
                    start=True,
                    stop=True,
                )
            s = small.tile([128, KC], f32, tag="s_sb")
            for k in range(KC):
                nc.scalar.activation(
                    s[:, k : k + 1],
                    sp[:, k : k + 1],
                    mybir.ActivationFunctionType.Sigmoid,
                    bias=b2_sb[:, k : k + 1],
                )

            for k in range(KC):
                nc.scalar.mul(xt[:, k, :], xt[:, k, :], s[:, k : k + 1])

            nc.sync.dma_start(
                out=out[b].rearrange("(k p) j -> p k j", p=128), in_=xt
            )

    nc.compile()
    return nc


def _get_nc():
    if "nc" not in _NC_CACHE:
        _NC_CACHE["nc"] = _build_nc()
    return _NC_CACHE["nc"]


def kernel(x, w1, b1, w2, b2):
    global LAST_RESULT
    from concourse.bass_utils import run_bass_kernel_spmd

    xf = np.ascontiguousarray(x.reshape(B, C, HW), dtype=np.float32)
    w1t = np.ascontiguousarray(w1.reshape(32, KC, 128).transpose(2, 1, 0))
    b1c = np.ascontiguousarray(b1.reshape(32, 1))
    w2t = np.ascontiguousarray(w2.reshape(KC, 128, 32).transpose(2, 0, 1))
    b2c = np.ascontiguousarray(b2.reshape(KC, 128).T)

    in_maps = [
        {
            "x": np.ascontiguousarray(xf[i * B_LOC : (i + 1) * B_LOC]),
            "w1t": w1t,
            "b1": b1c,
            "w2t": w2t,
            "b2c": b2c,
        }
        for i in range(N_CORES)
    ]

    nc = _get_nc()
    res = run_bass_kernel_spmd(
        nc, in_maps, core_ids=list(range(N_CORES)), trace=TRACE
    )
    LAST_RESULT = res
    out = np.concatenate([r["out"] for r in res.results], axis=0)
    return out.reshape(B, C, 64, 64)


# revision 3
# speedup vs baseline: 1.0445x; 1.0445x over previous
"""Squeeze-and-Excitation attention module on 8 Trainium2 NeuronCores.

Reference computation (per image b):
    y[c]  = mean(x[b, c, :, :])                      # global average pool
    z     = relu(w1 @ y + b1)                        # FC 512 -> 32
    s     = sigmoid(w2 @ z + b2)                     # FC 32 -> 512
    out[b, c, :, :] = x[b, c, :, :] * s[c]

Sharding: data-parallel over batch. 32 images / 8 cores = 4 images per
core; the tiny FC weights are replicated. Each core streams its images
through SBUF once (load -> pool -> FCs -> in-place scale -> store), so
HBM traffic is the 2x33.5 MB minimum per core.

Layouts (prepared host-side, all free for 68 KB of weights):
    x      [4, 512, 4096]  per-core shard, spatial flattened
    w1t    [128, 4, 32]    w1t[p, k, r] = w1[r, 128k + p]
    b1     [32, 1]
    w2t    [32, 4, 128]    w2t[r, k, p] = w2[128k + p, r]
    b2c    [128, 4]        b2c[p, k]   = b2[128k + p]

Per image, channels are split into 4 chunks of 128 (one SBUF tile
[128, 4096] = 2 MB each). Loads go out on the Sync HWDGE queue and
stores on the GpSimd SWDGE queue so a store waiting on compute never
head-of-line-blocks the next image's loads. Pooling reduces are split
between DVE (tensor_reduce) and ACT (in-place Copy + accum_out); the
scale multiplies are split the other way. Emission is software-
pipelined one image deep: image b's scale/store instructions are
emitted after image b+1's load/reduce so neither engine stream stalls
on the other image's dependencies.
"""

import numpy as np

B = 32
C = 512
HW = 64 * 64
N_CORES = 8
B_LOC = B // N_CORES
KC = C // 128  # channel chunks of 128

_NC_CACHE = {}

# Set by test harness to capture a profile; harmless default for grading.
TRACE = False
LAST_RESULT = None


def _build_nc():
    from contextlib import ExitStack

    import concourse.tile as tile
    from concourse import bacc, mybir

    f32 = mybir.dt.float32
    AF = mybir.ActivationFunctionType
    nc = bacc.Bacc("TRN2", target_bir_lowering=False, debug=False)

    x = nc.dram_tensor("x", [B_LOC, KC, 128, HW], f32, kind="ExternalInput")
    w1t = nc.dram_tensor("w1t", [128, KC, 32], f32, kind="ExternalInput")
    b1 = nc.dram_tensor("b1", [32, 1], f32, kind="ExternalInput")
    w2t = nc.dram_tensor("w2t", [32, KC, 128], f32, kind="ExternalInput")
    b2c = nc.dram_tensor("b2c", [128, KC], f32, kind="ExternalInput")
    out = nc.dram_tensor("out", [B_LOC, KC, 128, HW], f32, kind="ExternalOutput")

    with ExitStack() as ctx:
        tc = ctx.enter_context(tile.TileContext(nc))
        singles = ctx.enter_context(tc.tile_pool(name="singles", bufs=1))
        xpool = ctx.enter_context(tc.tile_pool(name="xpool", bufs=10))
        small = ctx.enter_context(tc.tile_pool(name="small", bufs=2))
        psum = ctx.enter_context(tc.tile_pool(name="psum", bufs=2, space="PSUM"))

        w1t_sb = singles.tile([128, KC, 32], f32)
        nc.sync.dma_start(out=w1t_sb, in_=w1t[:])
        b1_sb = singles.tile([32, 1], f32)
        nc.sync.dma_start(out=b1_sb, in_=b1[:])
        w2t_sb = singles.tile([32, KC, 128], f32)
        nc.sync.dma_start(out=w2t_sb, in_=w2t[:])
        b2_sb = singles.tile([128, KC], f32)
        nc.sync.dma_start(out=b2_sb, in_=b2c[:])

        # Engine split per image: pooling chunks 0-1 on DVE, 2-3 on ACT;
        # scale chunks 0-2 on DVE, 3 on ACT.
        REDUCE_DVE = (0, 1)
        MUL_ACT = (3,)

        prev = None  # (xts, s_tiles) of the previous image, not yet scaled

        def emit_scale_and_store(state, b):
            xts, s_tiles = state
            for k in range(KC):
                if k in MUL_ACT:
                    nc.scalar.mul(xts[k], xts[k], s_tiles[k])
                else:
                    nc.vector.tensor_scalar_mul(xts[k], xts[k], s_tiles[k])
            for k in range(KC):
                nc.gpsimd.dma_start(out=out[b, k], in_=xts[k])

        for b in range(B_LOC):
            xts = []
            for k in range(KC):
                xt = xpool.tile([128, HW], f32, tag="x")
                nc.sync.dma_start(out=xt, in_=x[b, k])
                xts.append(xt)

            zp = psum.tile([32, 1], f32, tag="z")
            for k in range(KC):
                sums = small.tile([128, 1], f32, tag=f"sum{k}")
                if k in REDUCE_DVE:
                    nc.vector.tensor_reduce(
                        out=sums,
                        in_=xts[k],
                        axis=mybir.AxisListType.X,
                        op=mybir.AluOpType.add,
                    )
                else:
                    nc.scalar.activation(
                        xts[k], xts[k], AF.Copy, accum_out=sums
                    )
                nc.tensor.matmul(
                    zp,
                    lhsT=w1t_sb[:, k, :],
                    rhs=sums,
                    start=(k == 0),
                    stop=(k == KC - 1),
                )

            # Previous image's scale+store goes here so its engine slots
            # sit behind this image's reduces, not in front of them.
            if prev is not None:
                emit_scale_and_store(prev, b - 1)

            z = small.tile([32, 1], f32, tag="z_sb")
            nc.scalar.activation(z, zp, AF.Relu, bias=b1_sb, scale=1.0 / HW)

            sp = psum.tile([128, KC], f32, tag="s")
            s_tiles = []
            for k in range(KC):
                nc.tensor.matmul(
                    sp[:, k : k + 1],
                    lhsT=w2t_sb[:, k, :],
                    rhs=z,
                    start=True,
                    stop=True,
                )
            for k in range(KC):
                s = small.tile([128, 1], f32, tag=f"s{k}")
                nc.scalar.activation(
                    s, sp[:, k : k + 1], AF.Sigmoid, bias=b2_sb[:, k : k + 1]
                )
                s_tiles.append(s)

            prev = (xts, s_tiles)

        emit_scale_and_store(prev, B_LOC - 1)

    nc.compile()
    return nc


def _get_nc():
    if "nc" not in _NC_CACHE:
        _NC_CACHE["nc"] = _build_nc()
    return _NC_CACHE["nc"]


def kernel(x, w1, b1, w2, b2):
    global LAST_RESULT
    from concourse.bass_utils import run_bass_kernel_spmd

    xf = np.ascontiguousarray(x.reshape(B, KC, 128, HW), dtype=np.float32)
    w1t = np.ascontiguousarray(w1.reshape(32, KC, 128).transpose(2, 1, 0))
    b1c = np.ascontiguousarray(b1.reshape(32, 1))
    w2t = np.ascontiguousarray(w2.reshape(KC, 128, 32).transpose(2, 0, 1))
    b2c = np.ascontiguousarray(b2.reshape(KC, 128).T)

    in_maps = [
        {
            "x": np.ascontiguousarray(xf[i * B_LOC : (i + 1) * B_LOC]),
            "w1t": w1t,
            "b1": b1c,
            "w2t": w2t,
            "b2c": b2c,
        }
        for i in range(N_CORES)
    ]

    nc = _get_nc()
    res = run_bass_kernel_spmd(
        nc, in_maps, core_ids=list(range(N_CORES)), trace=TRACE
    )
    LAST_RESULT = res
    out = np.concatenate([r["out"] for r in res.results], axis=0)
    return out.reshape(B, C, 64, 64)


# revision 4
# speedup vs baseline: 1.3363x; 1.2793x over previous
"""Squeeze-and-Excitation attention module on 8 Trainium2 NeuronCores.

Reference computation (per image b):
    y[c]  = mean(x[b, c, :, :])                      # global average pool
    z     = relu(w1 @ y + b1)                        # FC 512 -> 32
    s     = sigmoid(w2 @ z + b2)                     # FC 32 -> 512
    out[b, c, :, :] = x[b, c, :, :] * s[c]

Sharding: data-parallel over batch. 32 images / 8 cores = 4 images per
core; the tiny FC weights are replicated. Each core streams its images
through SBUF once (load -> pool -> FCs -> in-place scale -> store), so
HBM traffic is the 2x33.5 MB minimum per core.

Layouts (prepared host-side, all free for 68 KB of weights):
    x      [4, 512, 4096]  per-core shard, spatial flattened
    w1t    [128, 4, 32]    w1t[p, k, r] = w1[r, 128k + p]
    b1     [32, 1]
    w2t    [32, 4, 128]    w2t[r, k, p] = w2[128k + p, r]
    b2c    [128, 4]        b2c[p, k]   = b2[128k + p]

Per image, channels are split into 4 chunks of 128 (one SBUF tile
[128, 4096] = 2 MB each). Loads go out on the Sync HWDGE queue and
stores on the GpSimd SWDGE queue so a store waiting on compute never
head-of-line-blocks the next image's loads. Pooling reduces are split
between DVE (tensor_reduce) and ACT (in-place Copy + accum_out); the
scale multiplies are split the other way. Emission is software-
pipelined one image deep: image b's scale/store instructions are
emitted after image b+1's load/reduce so neither engine stream stalls
on the other image's dependencies.
"""

import numpy as np

B = 32
C = 512
HW = 64 * 64
N_CORES = 8
B_LOC = B // N_CORES
KC = C // 128  # channel chunks of 128

_NC_CACHE = {}

# Set by test harness to capture a profile; harmless default for grading.
TRACE = False
LAST_RESULT = None


def _build_nc():
    from contextlib import ExitStack

    import concourse.tile as tile
    from concourse import bacc, mybir

    f32 = mybir.dt.float32
    AF = mybir.ActivationFunctionType
    nc = bacc.Bacc("TRN2", target_bir_lowering=False, debug=False)

    x = nc.dram_tensor("x", [B_LOC, KC, 128, HW], f32, kind="ExternalInput")
    w1t = nc.dram_tensor("w1t", [128, KC, 32], f32, kind="ExternalInput")
    b1 = nc.dram_tensor("b1", [32, 1], f32, kind="ExternalInput")
    w2t = nc.dram_tensor("w2t", [32, KC, 128], f32, kind="ExternalInput")
    b2c = nc.dram_tensor("b2c", [128, KC], f32, kind="ExternalInput")
    out = nc.dram_tensor("out", [B_LOC, KC, 128, HW], f32, kind="ExternalOutput")

    with ExitStack() as ctx:
        tc = ctx.enter_context(tile.TileContext(nc))
        singles = ctx.enter_context(tc.tile_pool(name="singles", bufs=1))
        xpool = ctx.enter_context(tc.tile_pool(name="xpool", bufs=10))
        small = ctx.enter_context(tc.tile_pool(name="small", bufs=2))
        psum = ctx.enter_context(tc.tile_pool(name="psum", bufs=2, space="PSUM"))

        w1t_sb = singles.tile([128, KC, 32], f32)
        b1_sb = singles.tile([32, 1], f32)
        w2t_sb = singles.tile([32, KC, 128], f32)
        b2_sb = singles.tile([128, KC], f32)

        for b in range(B_LOC):
            xts = []
            for k in range(KC):
                xt = xpool.tile([128, HW], f32, tag="x")
                nc.sync.dma_start(out=xt, in_=x[b, k])
                xts.append(xt)

            if b == 0:
                # Weight loads sit behind the first image on the load
                # queue; they are only needed once pooling finishes.
                nc.sync.dma_start(out=w1t_sb, in_=w1t[:])
                nc.sync.dma_start(out=b1_sb, in_=b1[:])
                nc.sync.dma_start(out=w2t_sb, in_=w2t[:])
                nc.sync.dma_start(out=b2_sb, in_=b2c[:])

            zp = psum.tile([32, 1], f32, tag="z")
            for k in range(KC):
                sums = small.tile([128, 1], f32, tag=f"sum{k}")
                nc.vector.tensor_reduce(
                    out=sums,
                    in_=xts[k],
                    axis=mybir.AxisListType.X,
                    op=mybir.AluOpType.add,
                )
                nc.tensor.matmul(
                    zp,
                    lhsT=w1t_sb[:, k, :],
                    rhs=sums,
                    start=(k == 0),
                    stop=(k == KC - 1),
                )

            z = small.tile([32, 1], f32, tag="z_sb")
            nc.scalar.activation(z, zp, AF.Relu, bias=b1_sb, scale=1.0 / HW)

            sp = psum.tile([128, KC], f32, tag="s")
            s_tiles = []
            for k in range(KC):
                nc.tensor.matmul(
                    sp[:, k : k + 1],
                    lhsT=w2t_sb[:, k, :],
                    rhs=z,
                    start=True,
                    stop=True,
                )
            for k in range(KC):
                s = small.tile([128, 1], f32, tag=f"s{k}")
                nc.scalar.activation(
                    s, sp[:, k : k + 1], AF.Sigmoid, bias=b2_sb[:, k : k + 1]
                )
                s_tiles.append(s)

            # Scale in place and store. ACT handles the multiplies (DVE
            # stays dedicated to pooling so it never blocks on this
            # image's sigmoid); the last image splits them across both
            # engines to shorten the drain tail.
            last = b == B_LOC - 1
            for k in range(KC):
                if last and k >= 2:
                    nc.vector.tensor_scalar_mul(xts[k], xts[k], s_tiles[k])
                else:
                    nc.scalar.mul(xts[k], xts[k], s_tiles[k])
                nc.gpsimd.dma_start(out=out[b, k], in_=xts[k])

    nc.compile()
    return nc


def _get_nc():
    if "nc" not in _NC_CACHE:
        _NC_CACHE["nc"] = _build_nc()
    return _NC_CACHE["nc"]


def kernel(x, w1, b1, w2, b2):
    global LAST_RESULT
    from concourse.bass_utils import run_bass_kernel_spmd

    xf = np.ascontiguousarray(x.reshape(B, KC, 128, HW), dtype=np.float32)
    w1t = np.ascontiguousarray(w1.reshape(32, KC, 128).transpose(2, 1, 0))
    b1c = np.ascontiguousarray(b1.reshape(32, 1))
    w2t = np.ascontiguousarray(w2.reshape(KC, 128, 32).transpose(2, 0, 1))
    b2c = np.ascontiguousarray(b2.reshape(KC, 128).T)

    in_maps = [
        {
            "x": np.ascontiguousarray(xf[i * B_LOC : (i + 1) * B_LOC]),
            "w1t": w1t,
            "b1": b1c,
            "w2t": w2t,
            "b2c": b2c,
        }
        for i in range(N_CORES)
    ]

    nc = _get_nc()
    res = run_bass_kernel_spmd(
        nc, in_maps, core_ids=list(range(N_CORES)), trace=TRACE
    )
    LAST_RESULT = res
    out = np.concatenate([r["out"] for r in res.results], axis=0)
    return out.reshape(B, C, 64, 64)
